# revision 4
# baseline (speedup 1.0000x reference)
"""Trainium2 Bass kernel for nn_DistanceAwareSelfAttentionHead.

Math (reference):
    s  = sigmoid((edge_attr - ib) * im)                    [E]
    rk = Ek1 + s*dEk ; rq = Eq1 + s*dEq ; rv = Ev1 + s*dEv (per edge, rank-1 in s)
    k = x@Wk ; q = x@Wq ; v = x@Wv
    A  = 2 q k^T ; A[src,dst] += q[src].rk + k[dst].rq     (duplicate edges summed)
    P  = softmax(A / sqrt(512))
    M  = P v + segsum(P[src,dst] * rv, src)

Key identities used (dense variant):
    q[src].rk = a1[src] + s*a2[src]   with a1 = q@Ek1, a2 = q@dEk
    k[dst].rq = b1[dst] + s*b2[dst]   with b1 = k@Eq1, b2 = k@dEq
    Define dense per-cell planes (host-precomputed, pure input functions):
        Mm[i,j] = #edges (i->j),  Sm[i,j] = sum of s over edges (i->j)
    Then the summed edge bias is DENSE:
        B = Mm * (a1[i] + b1[j]) + Sm * (a2[i] + b2[j])
    and the A*Rv segment term row i is u1[i]*Ev1 + u2[i]*dEv with
        u1 = rowsum(P .* Mm),  u2 = rowsum(P .* Sm).
    softmax without max-subtraction (logits bounded ~|12|), normalize at end.

Sharding: rows of A/q/M split across 8 cores (512 rows each); k, v, params
replicated (recomputed per core, no collectives).

This dense formulation needs NO gpsimd indirect ops (the HW bottleneck of
the previous sparse union-slot design): the bias is built with ACT
(Identity + per-partition bias) and DVE elementwise ops; u1/u2 are dense
multiply+reduce. Only act table is exp_and_others (Identity+Exp+Copy).
"""

import sys

if "/opt/trn_rl_repo" not in sys.path:
    sys.path.insert(0, "/opt/trn_rl_repo")

from contextlib import ExitStack

import numpy as np

import concourse.bacc as bacc
import concourse.mybir as mybir
import concourse.tile as tile
from concourse.bass_utils import run_bass_kernel_spmd

try:
    import ml_dtypes

    BF16_NP = ml_dtypes.bfloat16
except Exception:  # pragma: no cover
    BF16_NP = None

# ---- problem constants (hardcoded per the harness contract) ----
N = 4096
FEAT = 512
HID = 256
NCORES = 8
RPC = N // NCORES  # 512 rows per core
RB = RPC // 128  # 4 row blocks
SCALE = 1.0 / np.sqrt(np.float32(FEAT))

F32 = mybir.dt.float32
F32R = mybir.dt.float32r
BF16 = mybir.dt.bfloat16

_CACHE = {}


def _build_nc(reps=1):
    PD = BF16
    nc = bacc.Bacc(
        "TRN2",
        target_bir_lowering=False,
        debug=False,
        enable_asserts=False,
        num_devices=NCORES,
    )
    d = {}

    def din(name, shape, dtype=F32):
        d[name] = nc.dram_tensor(name, shape, dtype, kind="ExternalInput").ap()

    din("xt", [4, 128, N], F32R)  # x^T, feat-major chunks
    din("xtloc", [4, 128, RPC], F32R)  # local columns of x^T
    din("wk", [4, 128, HID], F32R)
    din("wq2", [4, 128, HID], F32R)  # 2*Wq
    din("wv", [4, 128, FEAT], F32R)
    din("ekbh", [2, 128, 2], F32R)  # [Ek1 | dEk] / 2  (chunked over HID)
    din("eqb", [2, 128, 2], F32R)  # [Eq1 | dEq]
    din("ev1bc", [128, FEAT])  # Ev1 broadcast
    din("devbc", [128, FEAT])  # dEv broadcast
    din("ident", [128, 128], PD)
    din("ones1", [1, 128], BF16)
    din("mmat", [RB, 128, N], BF16)  # edge multiplicity per cell
    din("smat", [RB, 128, N], BF16)  # sum of sigmoids per cell
    mout = nc.dram_tensor("mloc", [RPC, FEAT], F32, kind="ExternalOutput").ap()

    AF = mybir.ActivationFunctionType
    OP = mybir.AluOpType

    with tile.TileContext(nc) as tc:
      for _rep in range(reps):
        with ExitStack() as ctx:
            cpool = ctx.enter_context(tc.tile_pool(name="consts", bufs=1))
            wk_t = cpool.tile([128, 4, HID], F32R)
            wq2_t = cpool.tile([128, 4, HID], F32R)
            ekbh_t = cpool.tile([128, 2, 2], F32R)
            eqb_t = cpool.tile([128, 2, 2], F32R)
            ones1_t = cpool.tile([1, 128], BF16)
            ident_t = cpool.tile([128, 128], PD)
            sm_t = cpool.tile([128, 24], F32)  # zA|zB|rz|u1|u2 per row-block
            a12_t = cpool.tile([128, RB, 2], F32)
            ev1bc_t = cpool.tile([128, FEAT], F32)
            devbc_t = cpool.tile([128, FEAT], F32)

            for kc in range(4):
                nc.sync.dma_start(wk_t[:, kc, :], d["wk"][kc])
                nc.sync.dma_start(wq2_t[:, kc, :], d["wq2"][kc])
            for hg in range(2):
                nc.sync.dma_start(ekbh_t[:, hg, :], d["ekbh"][hg])
                nc.sync.dma_start(eqb_t[:, hg, :], d["eqb"][hg])
            nc.sync.dma_start(ones1_t[:], d["ones1"][:])
            nc.sync.dma_start(ident_t[:], d["ident"][:])
            nc.sync.dma_start(ev1bc_t[:], d["ev1bc"][:])
            nc.sync.dma_start(devbc_t[:], d["devbc"][:])

            with tc.tile_pool(name="mid", bufs=1) as pmid:
                qt2_t = pmid.tile([128, 2, RPC], F32R)
                kt_t = pmid.tile([128, 2, N], F32R)
                b1bc_t = pmid.tile([128, N], BF16)
                b2bc_t = pmid.tile([128, N], BF16)
                v_t = pmid.tile([128, 32, FEAT], PD)
                expa_t = pmid.tile([128, RB, N], PD)

                # ---------- phase 1: kT, qT2, v (x^T resident once) --------
                with tc.tile_pool(name="ph1", bufs=1) as p1, tc.tile_pool(
                    name="ps1", bufs=1, space="PSUM"
                ) as ps1:
                    xt_t = p1.tile([128, 4, N], F32R)
                    xtloc_t = p1.tile([128, 4, RPC], F32R)
                    for kc in range(4):
                        nc.sync.dma_start(xtloc_t[:, kc, :], d["xtloc"][kc])
                    # xt arrives in 4 column blocks of 1024 so PE starts early
                    for j in range(4):
                        nc.sync.dma_start(
                            xt_t[:, :, j * 1024 : (j + 1) * 1024],
                            d["xt"][:, :, j * 1024 : (j + 1) * 1024].rearrange(
                                "c p f -> p c f"
                            ),
                        )
                        for hg in range(2):
                            for n in range(2 * j, 2 * j + 2):
                                kps = ps1.tile([128, 512], F32, tag="kps", bufs=2)
                                for kc in range(4):
                                    nc.tensor.matmul(
                                        kps[:],
                                        wk_t[:, kc, hg * 128 : (hg + 1) * 128],
                                        xt_t[:, kc, n * 512 : (n + 1) * 512],
                                        start=(kc == 0),
                                        stop=(kc == 3),
                                    )
                                nc.scalar.copy(
                                    kt_t[:, hg, n * 512 : (n + 1) * 512], kps[:]
                                )
                    for hg in range(2):
                        qps = ps1.tile([128, 512], F32, tag="qps", bufs=2)
                        for kc in range(4):
                            nc.tensor.matmul(
                                qps[:],
                                wq2_t[:, kc, hg * 128 : (hg + 1) * 128],
                                xtloc_t[:, kc, :],
                                start=(kc == 0),
                                stop=(kc == 3),
                            )
                        nc.vector.tensor_copy(qt2_t[:, hg, :], qps[:])
                    for half in range(2):
                        wv_t = p1.tile([128, 4, FEAT // 2], F32R, tag="wvh")
                        for kc in range(4):
                            nc.sync.dma_start(
                                wv_t[:, kc, :],
                                d["wv"][kc][:, half * 256 : (half + 1) * 256],
                            )
                        for mg in range(32):
                            vps = ps1.tile([128, FEAT // 2], F32, tag="vps", bufs=4)
                            for kc in range(4):
                                nc.tensor.matmul(
                                    vps[:],
                                    xt_t[:, kc, mg * 128 : (mg + 1) * 128],
                                    wv_t[:, kc, :],
                                    start=(kc == 0),
                                    stop=(kc == 3),
                                )
                            nc.scalar.copy(
                                v_t[:, mg, half * 256 : (half + 1) * 256], vps[:]
                            )

                # ---------- phase 2a: b tables (bcast) + a12 ----------------
                with tc.tile_pool(name="ph2a", bufs=1) as p2, tc.tile_pool(
                    name="ps2a", bufs=1, space="PSUM"
                ) as ps2:
                    for row, dst_t in ((0, b1bc_t), (1, b2bc_t)):
                        for n in range(8):
                            bps = ps2.tile([1, 512], F32, tag="bps", bufs=2)
                            for hg in range(2):
                                nc.tensor.matmul(
                                    bps[:],
                                    eqb_t[:, hg, row : row + 1],
                                    kt_t[:, hg, n * 512 : (n + 1) * 512],
                                    start=(hg == 0),
                                    stop=(hg == 1),
                                )
                            brow = p2.tile([1, 512], BF16, tag="brow", bufs=2)
                            nc.vector.tensor_copy(brow[:], bps[:])
                            cps = ps2.tile([128, 512], F32, tag="cps", bufs=2)
                            nc.tensor.matmul(
                                cps[:], ones1_t[:], brow[:], start=True, stop=True
                            )
                            nc.scalar.copy(
                                dst_t[:, n * 512 : (n + 1) * 512], cps[:]
                            )
                    for mg in range(RB):
                        aps = ps2.tile([128, 2], F32, tag="aps", bufs=1)
                        for hg in range(2):
                            nc.tensor.matmul(
                                aps[:],
                                qt2_t[:, hg, mg * 128 : (mg + 1) * 128],
                                ekbh_t[:, hg, :],
                                start=(hg == 0),
                                stop=(hg == 1),
                            )
                        nc.vector.tensor_copy(a12_t[:, mg, :], aps[:])

                # ---------- phase 2b: per row-block dense bias + A + exp + M
                with tc.tile_pool(name="edge", bufs=2) as ep, tc.tile_pool(
                    name="ps2b", bufs=1, space="PSUM"
                ) as psb:
                    for rb in range(RB):
                        mm = ep.tile([128, N], BF16, tag="mm")
                        sg = ep.tile([128, N], BF16, tag="sg")
                        nc.sync.dma_start(mm[:], d["mmat"][rb])
                        nc.sync.dma_start(sg[:], d["smat"][rb])

                        # ac = 2qk^T + Mm*(a1+b1) + Sm*(a2+b2), per 512-col blk
                        ac = ep.tile([128, N], F32, tag="ac", bufs=1)
                        for q in range(8):
                            s0, s1 = q * 512, (q + 1) * 512
                            aps2 = psb.tile([128, 512], F32, tag="apsA", bufs=3)
                            for hg in range(2):
                                nc.tensor.matmul(
                                    aps2[:],
                                    qt2_t[:, hg, rb * 128 : (rb + 1) * 128],
                                    kt_t[:, hg, s0:s1],
                                    start=(hg == 0),
                                    stop=(hg == 1),
                                )
                            u = ep.tile([128, 512], BF16, tag="u", bufs=2)
                            w = ep.tile([128, 512], BF16, tag="w", bufs=2)
                            nc.scalar.activation(
                                u[:], b1bc_t[:, s0:s1], AF.Identity,
                                bias=a12_t[:, rb, 0:1],
                            )
                            nc.scalar.activation(
                                w[:], b2bc_t[:, s0:s1], AF.Identity,
                                bias=a12_t[:, rb, 1:2],
                            )
                            nc.vector.tensor_tensor(u[:], u[:], mm[:, s0:s1], OP.mult)
                            nc.vector.tensor_tensor(w[:], w[:], sg[:, s0:s1], OP.mult)
                            nc.vector.tensor_tensor(u[:], u[:], w[:], OP.add)
                            nc.vector.tensor_tensor(
                                ac[:, s0:s1], aps2[:], u[:], OP.add
                            )

                        # exp halves with Z accumulation
                        nc.scalar.activation(
                            expa_t[:, rb, 0 : N // 2],
                            ac[:, 0 : N // 2],
                            AF.Exp,
                            scale=float(SCALE),
                            accum_out=sm_t[:, rb : rb + 1],
                        )
                        nc.scalar.activation(
                            expa_t[:, rb, N // 2 : N],
                            ac[:, N // 2 : N],
                            AF.Exp,
                            scale=float(SCALE),
                            accum_out=sm_t[:, 4 + rb : 5 + rb],
                        )
                        nc.vector.tensor_tensor(
                            sm_t[:, 8 + rb : 9 + rb],
                            sm_t[:, rb : rb + 1],
                            sm_t[:, 4 + rb : 5 + rb],
                            OP.add,
                        )
                        nc.vector.reciprocal(
                            sm_t[:, 8 + rb : 9 + rb], sm_t[:, 8 + rb : 9 + rb]
                        )

                        # u1 = rowsum(P .* Mm) ; u2 = rowsum(P .* Sm)
                        upr = ep.tile([128, N], BF16, tag="upr", name="upr", bufs=1)
                        nc.vector.tensor_tensor(
                            upr[:], mm[:], expa_t[:, rb, :], OP.mult
                        )
                        nc.vector.tensor_reduce(
                            sm_t[:, 12 + rb : 13 + rb], upr[:],
                            mybir.AxisListType.X, OP.add,
                        )
                        upr2 = ep.tile([128, N], BF16, tag="upr", name="upr2", bufs=1)
                        nc.vector.tensor_tensor(
                            upr2[:], sg[:], expa_t[:, rb, :], OP.mult
                        )
                        nc.vector.tensor_reduce(
                            sm_t[:, 16 + rb : 17 + rb], upr2[:],
                            mybir.AxisListType.X, OP.add,
                        )

                        # transposes + M matmuls for this row block
                        mps = psb.tile([128, FEAT], F32, tag="mps", bufs=2)
                        for cg in range(8):
                            tp = psb.tile([128, 512], PD, tag="tp", bufs=3)
                            for j in range(4):
                                c = 4 * cg + j
                                nc.tensor.transpose(
                                    tp[:, j * 128 : (j + 1) * 128],
                                    expa_t[:, rb, c * 128 : (c + 1) * 128],
                                    ident_t[:],
                                )
                            pt = ep.tile([128, 512], PD, tag="pt", bufs=3)
                            if cg % 2 == 0:
                                nc.vector.tensor_copy(pt[:], tp[:])
                            else:
                                nc.scalar.copy(pt[:], tp[:])
                            for j in range(4):
                                c = 4 * cg + j
                                nc.tensor.matmul(
                                    mps[:],
                                    pt[:, j * 128 : (j + 1) * 128],
                                    v_t[:, c, :],
                                    start=(c == 0),
                                    stop=(c == 31),
                                )
                        # combine: (mps + u1*Ev1 + u2*dEv) * rz -> out
                        t1 = ep.tile([128, FEAT], F32, tag="t1", bufs=1)
                        t2 = ep.tile([128, FEAT], F32, tag="t2", bufs=1)
                        mf = ep.tile([128, FEAT], F32, tag="mf", bufs=2)
                        nc.vector.tensor_scalar(
                            t1[:], ev1bc_t[:], sm_t[:, 12 + rb : 13 + rb],
                            None, OP.mult,
                        )
                        nc.vector.tensor_scalar(
                            t2[:], devbc_t[:], sm_t[:, 16 + rb : 17 + rb],
                            None, OP.mult,
                        )
                        nc.vector.tensor_tensor(t1[:], t1[:], t2[:], OP.add)
                        nc.vector.tensor_tensor(t1[:], t1[:], mps[:], OP.add)
                        nc.vector.tensor_scalar(
                            mf[:], t1[:], sm_t[:, 8 + rb : 9 + rb], None, OP.mult
                        )
                        nc.sync.dma_start(mout[rb * 128 : (rb + 1) * 128, :], mf[:])

    nc.compile()
    return nc


def _prep(inputs):
    x = np.asarray(inputs["x"], np.float32)
    ei = np.asarray(inputs["edge_index"]).astype(np.int64)
    ea = np.asarray(inputs["edge_attr"], np.float32).reshape(-1)
    Wk = np.asarray(inputs["Wk"], np.float32)
    Wq = np.asarray(inputs["Wq"], np.float32)
    Wv = np.asarray(inputs["Wv"], np.float32)
    Ek = np.asarray(inputs["Ek"], np.float32)
    Eq = np.asarray(inputs["Eq"], np.float32)
    Ev = np.asarray(inputs["Ev"], np.float32)
    ib = float(np.asarray(inputs["idx_bias"]).reshape(()))
    im = float(np.asarray(inputs["idx_mult"]).reshape(()))

    src, dst = ei[0], ei[1]
    assert BF16_NP is not None, "ml_dtypes needed"

    # dense per-cell multiplicity and sigmoid-sum planes (duplicates summed)
    sgm = 1.0 / (1.0 + np.exp(-(ea - ib) * im))
    lin = src * N + dst
    mfull = np.bincount(lin, minlength=N * N).astype(np.float32)
    sfull = np.bincount(lin, weights=sgm, minlength=N * N).astype(np.float32)
    mfull = mfull.reshape(N, N)
    sfull = sfull.reshape(N, N)

    xT = np.ascontiguousarray(x.T)  # [FEAT, N]
    ident_np = np.eye(128, dtype=np.float32).astype(BF16_NP)
    shared = {
        "xt": np.ascontiguousarray(xT.reshape(4, 128, N)),
        "wk": np.ascontiguousarray(Wk.reshape(4, 128, HID)),
        "wq2": np.ascontiguousarray((2.0 * Wq).reshape(4, 128, HID)),
        "wv": np.ascontiguousarray(Wv.reshape(4, 128, FEAT)),
        "ekbh": np.ascontiguousarray(
            (0.5 * np.stack([Ek[1], Ek[0] - Ek[1]], axis=1)).reshape(2, 128, 2)
        ),
        "eqb": np.ascontiguousarray(
            np.stack([Eq[1], Eq[0] - Eq[1]], axis=1).reshape(2, 128, 2)
        ),
        "ev1bc": np.ascontiguousarray(np.broadcast_to(Ev[1], (128, FEAT))),
        "devbc": np.ascontiguousarray(np.broadcast_to(Ev[0] - Ev[1], (128, FEAT))),
        "ident": ident_np,
        "ones1": np.ones((1, 128), BF16_NP),
    }
    in_maps = []
    for cc in range(NCORES):
        m = dict(shared)
        m["xtloc"] = np.ascontiguousarray(
            xT[:, cc * RPC : (cc + 1) * RPC].reshape(4, 128, RPC)
        )
        m["mmat"] = np.ascontiguousarray(
            mfull[cc * RPC : (cc + 1) * RPC].reshape(RB, 128, N)
        ).astype(BF16_NP)
        m["smat"] = np.ascontiguousarray(
            sfull[cc * RPC : (cc + 1) * RPC].reshape(RB, 128, N)
        ).astype(BF16_NP)
        in_maps.append(m)
    return in_maps


def get_nc(reps=1):
    key = ("dense", reps)
    if key not in _CACHE:
        _CACHE[key] = _build_nc(reps)
    return _CACHE[key]


def kernel(**inputs) -> np.ndarray:
    nc = get_nc()
    in_maps = _prep(inputs)
    res = run_bass_kernel_spmd(nc, in_maps, list(range(NCORES)))
    return np.concatenate(
        [res.results[cc]["mloc"] for cc in range(NCORES)], axis=0
    ).astype(np.float32)


# revision 13
# speedup vs baseline: 1.9136x; 1.9136x over previous
"""Trainium2 Bass kernel for nn_DistanceAwareSelfAttentionHead (v2).

Math (reference):
    s  = sigmoid((edge_attr - ib) * im)                    [E]
    rk = Ek1 + s*dEk ; rq = Eq1 + s*dEq ; rv = Ev1 + s*dEv (per edge)
    k = x@Wk ; q = x@Wq ; v = x@Wv
    A  = 2 q k^T ; A[src,dst] += q[src].rk + k[dst].rq     (dup edges summed)
    P  = softmax(A / sqrt(512)) ;  M = P v + segsum(P[src,dst]*rv, src)

Identities:
    q[src].rk = a1[src] + s*a2[src],  a1 = q@Ek1, a2 = q@dEk
    k[dst].rq = b1[dst] + s*b2[dst],  b1 = k@Eq1, b2 = k@dEq
    Dense per-cell planes (host-built COO, device local_scatter):
        Mm[i,j] = #edges(i->j), Sm[i,j] = sum_s(i->j)
    bias  B = Mm*(a1+b1) + Sm*(a2+b2)          (dense elementwise)
    segsum row i = u1[i]*Ev1 + u2[i]*dEv, u1 = rowsum(P.*Mm), u2 = rowsum(P.*Sm)
    softmax without max-subtraction (logits bounded), normalize at the end.

Sharding (minimizes host->device input bytes, the dominant cost):
    * q-rows / output rows sharded 8 ways (512 rows per core).
    * x^T arrives ONLY as the local 512-column slice (bf16).
    * Wk|Wq|Wv arrive as a per-core 1/8 column shard; AllGather #1
      reassembles the full weights on every core.
    * each core computes k,v for its 512 nodes; AllGather #2 shares them.
    * edges partitioned by src; COO (dst, S, M) padded per (row, quadrant).

Per-core input bytes ~1.1MB vs 15.4MB for the reference-style kernel.
"""

import sys

if "/opt/trn_rl_repo" not in sys.path:
    sys.path.insert(0, "/opt/trn_rl_repo")

from contextlib import ExitStack

import numpy as np

import concourse.bacc as bacc
import concourse.mybir as mybir
import concourse.tile as tile
from concourse.bass_utils import run_bass_kernel_spmd

try:
    import ml_dtypes

    BF16_NP = ml_dtypes.bfloat16
except Exception:  # pragma: no cover
    BF16_NP = None

# ---- problem constants (hardcoded per the harness contract) ----
N = 4096
FEAT = 512
HID = 256
NCORES = 8
RPC = N // NCORES  # 512 rows per core
RB = RPC // 128  # 4 row blocks per core
WQ = 32  # padded cells per (row, dst-quadrant)
SCALE = 1.0 / np.sqrt(np.float32(FEAT))

F32 = mybir.dt.float32
BF16 = mybir.dt.bfloat16
I16 = mybir.dt.int16

_CACHE = {}


def _build_nc(reps=1):
    nc = bacc.Bacc(
        "TRN2",
        target_bir_lowering=False,
        debug=False,
        enable_asserts=False,
        num_devices=NCORES,
    )
    d = {}

    def din(name, shape, dtype):
        d[name] = nc.dram_tensor(name, shape, dtype, kind="ExternalInput").ap()

    din("xtloc", [4, 128, RPC], BF16)  # local x^T slice, feat-chunked
    din("wsh", [4, 128, 128], BF16)  # per-core shard of [Wk|Wq2|Wv] columns
    din("ekbh", [2, 128, 2], BF16)  # [Ek1 | dEk] / 2  (chunked over HID)
    din("eqb", [2, 128, 2], BF16)  # [Eq1 | dEq]
    din("evr", [2, FEAT], BF16)  # rows: Ev1 ; dEv
    din("ident", [128, 128], BF16)
    din("ones1", [1, 128], BF16)
    din("dste", [RB, 4, 128, WQ], I16)  # dst%1024 per cell, -1 pad
    din("se", [RB, 4, 128, WQ], BF16)  # sigmoid-sum per cell
    din("me", [RB, 4, 128, WQ], BF16)  # multiplicity per cell
    mout = nc.dram_tensor("mloc", [RPC, FEAT], F32, kind="ExternalOutput").ap()

    AF = mybir.ActivationFunctionType
    OP = mybir.AluOpType
    ALL = [list(range(NCORES))]

    with tile.TileContext(nc) as tc:
      for _rep in range(reps):
        with ExitStack() as ctx:
            cpool = ctx.enter_context(tc.tile_pool(name="consts", bufs=1))
            ekbh_t = cpool.tile([128, 2, 2], BF16)
            eqb_t = cpool.tile([128, 2, 2], BF16)
            ev1r_t = cpool.tile([1, FEAT], BF16)
            devr_t = cpool.tile([1, FEAT], BF16)
            ones1_t = cpool.tile([1, 128], BF16)
            ident_t = cpool.tile([128, 128], BF16)
            sm_t = cpool.tile([128, 24], F32)  # zA|zB|rz|u1|u2 per row-block
            a12_t = cpool.tile([128, RB, 2], F32)
            ev1bc_t = cpool.tile([128, FEAT], F32)
            devbc_t = cpool.tile([128, FEAT], F32)
            wall_t = cpool.tile([128, 4, 1024], BF16)  # [Wk|Wq2|Wv] full

            for hg in range(2):
                nc.sync.dma_start(ekbh_t[:, hg, :], d["ekbh"][hg])
                nc.sync.dma_start(eqb_t[:, hg, :], d["eqb"][hg])
            nc.sync.dma_start(ev1r_t[:], d["evr"][0:1])
            nc.sync.dma_start(devr_t[:], d["evr"][1:2])
            nc.sync.dma_start(ones1_t[:], d["ones1"][:])
            nc.sync.dma_start(ident_t[:], d["ident"][:])

            with tc.tile_pool(name="dram", bufs=1, space="DRAM") as dram:
                # ---- collective #1: AllGather the weight shards ----------
                wsin_d = dram.tile([4, 128, 128], BF16)
                wsout_d = dram.tile(
                    [NCORES, 4, 128, 128], BF16, addr_space="Shared"
                )
                nc.gpsimd.dma_start(wsin_d[:], d["wsh"][:])
                nc.gpsimd.collective_compute(
                    "AllGather",
                    OP.bypass,
                    replica_groups=ALL,
                    ins=[wsin_d.opt()],
                    outs=[wsout_d.opt()],
                )
                for c in range(NCORES):
                    nc.sync.dma_start(
                        wall_t[:, :, c * 128 : (c + 1) * 128],
                        wsout_d[c].rearrange("k p j -> p k j"),
                    )
                wk_t = wall_t[:, :, 0:HID]
                wq2_t = wall_t[:, :, HID : 2 * HID]
                wv_t = wall_t[:, :, 2 * HID :]

                with tc.tile_pool(name="mid", bufs=1) as pmid:
                    qt2_t = pmid.tile([128, 2, RPC], BF16)
                    kt_t = pmid.tile([128, 2, N], BF16)
                    b1bc_t = pmid.tile([128, N], BF16)
                    b2bc_t = pmid.tile([128, N], BF16)
                    v_t = pmid.tile([128, 32, FEAT], BF16)
                    expa_t = pmid.tile([128, RB, N], BF16)

                    # ---- phase 0: local k, q2, v from xtloc --------------
                    kvin_d = dram.tile([128, 6, RPC], BF16)
                    kvout_d = dram.tile(
                        [NCORES, 128, 6, RPC], BF16, addr_space="Shared"
                    )
                    with tc.tile_pool(name="ph0", bufs=1) as p0, tc.tile_pool(
                        name="ps0", bufs=1, space="PSUM"
                    ) as ps0:
                        xtloc_t = p0.tile([128, 4, RPC], BF16)
                        kloc_t = p0.tile([128, 2, RPC], BF16)
                        vloc_t = p0.tile([128, 4, RPC], BF16)
                        for kc in range(4):
                            nc.sync.dma_start(xtloc_t[:, kc, :], d["xtloc"][kc])
                        for hg in range(2):
                            kps = ps0.tile([128, RPC], F32, tag="kps", bufs=2)
                            for kc in range(4):
                                nc.tensor.matmul(
                                    kps[:],
                                    wk_t[:, kc, hg * 128 : (hg + 1) * 128],
                                    xtloc_t[:, kc, :],
                                    start=(kc == 0),
                                    stop=(kc == 3),
                                )
                            nc.scalar.copy(kloc_t[:, hg, :], kps[:])
                            qps = ps0.tile([128, RPC], F32, tag="kps", bufs=2)
                            for kc in range(4):
                                nc.tensor.matmul(
                                    qps[:],
                                    wq2_t[:, kc, hg * 128 : (hg + 1) * 128],
                                    xtloc_t[:, kc, :],
                                    start=(kc == 0),
                                    stop=(kc == 3),
                                )
                            nc.vector.tensor_copy(qt2_t[:, hg, :], qps[:])
                        for mg in range(4):
                            vps = ps0.tile([128, FEAT], F32, tag="vps", bufs=2)
                            for kc in range(4):
                                nc.tensor.matmul(
                                    vps[:],
                                    xtloc_t[:, kc, mg * 128 : (mg + 1) * 128],
                                    wv_t[:, kc, :],
                                    start=(kc == 0),
                                    stop=(kc == 3),
                                )
                            nc.scalar.copy(vloc_t[:, mg, :], vps[:])
                        # bounce local k|v to DRAM for the collective
                        nc.sync.dma_start(kvin_d[:, 0:2, :], kloc_t[:])
                        nc.sync.dma_start(kvin_d[:, 2:6, :], vloc_t[:])

                    # ---- collective #2: AllGather k|v --------------------
                    nc.gpsimd.collective_compute(
                        "AllGather",
                        OP.bypass,
                        replica_groups=ALL,
                        ins=[kvin_d.opt()],
                        outs=[kvout_d.opt()],
                    )
                    for c in range(NCORES):
                        nc.sync.dma_start(
                            kt_t[:, :, c * RPC : (c + 1) * RPC],
                            kvout_d[c, :, 0:2, :],
                        )
                        nc.sync.dma_start(
                            v_t[:, c * 4 : (c + 1) * 4, :],
                            kvout_d[c, :, 2:6, :],
                        )

                    # ---- phase 2a: b tables (bcast), a12, Ev rows --------
                    with tc.tile_pool(name="ph2a", bufs=1) as p2, tc.tile_pool(
                        name="ps2a", bufs=1, space="PSUM"
                    ) as ps2:
                        for row, dst_t in ((0, b1bc_t), (1, b2bc_t)):
                            for n in range(8):
                                bps = ps2.tile([1, 512], F32, tag="bps", bufs=2)
                                for hg in range(2):
                                    nc.tensor.matmul(
                                        bps[:],
                                        eqb_t[:, hg, row : row + 1],
                                        kt_t[:, hg, n * 512 : (n + 1) * 512],
                                        start=(hg == 0),
                                        stop=(hg == 1),
                                    )
                                brow = p2.tile([1, 512], BF16, tag="brow", bufs=2)
                                nc.vector.tensor_copy(brow[:], bps[:])
                                cps = ps2.tile([128, 512], F32, tag="cps", bufs=2)
                                nc.tensor.matmul(
                                    cps[:], ones1_t[:], brow[:],
                                    start=True, stop=True,
                                )
                                nc.scalar.copy(
                                    dst_t[:, n * 512 : (n + 1) * 512], cps[:]
                                )
                        for srt, tgt in ((ev1r_t, ev1bc_t), (devr_t, devbc_t)):
                            eps = ps2.tile([128, FEAT], F32, tag="cps", bufs=2)
                            nc.tensor.matmul(
                                eps[:], ones1_t[:], srt[:],
                                start=True, stop=True,
                            )
                            nc.scalar.copy(tgt[:], eps[:])
                        for mg in range(RB):
                            aps = ps2.tile([128, 2], F32, tag="aps", bufs=1)
                            for hg in range(2):
                                nc.tensor.matmul(
                                    aps[:],
                                    qt2_t[:, hg, mg * 128 : (mg + 1) * 128],
                                    ekbh_t[:, hg, :],
                                    start=(hg == 0),
                                    stop=(hg == 1),
                                )
                            nc.vector.tensor_copy(a12_t[:, mg, :], aps[:])

                    # ---- phase 2b: per row-block dense bias + A + exp + M
                    with tc.tile_pool(name="edge", bufs=2) as ep, tc.tile_pool(
                        name="ps2b", bufs=1, space="PSUM"
                    ) as psb:
                        for rb in range(RB):
                            ed = ep.tile([128, 4, WQ], I16, tag="ed")
                            es = ep.tile([128, 4, WQ], BF16, tag="es")
                            em = ep.tile([128, 4, WQ], BF16, tag="em")
                            nc.sync.dma_start(
                                ed[:], d["dste"][rb].rearrange("q p w -> p q w")
                            )
                            nc.sync.dma_start(
                                es[:], d["se"][rb].rearrange("q p w -> p q w")
                            )
                            nc.sync.dma_start(
                                em[:], d["me"][rb].rearrange("q p w -> p q w")
                            )
                            mm = ep.tile([128, N], BF16, tag="mm")
                            sg2 = ep.tile([128, N], BF16, tag="sg2")
                            for qd in range(4):
                                nc.gpsimd.local_scatter(
                                    mm[:, qd * 1024 : (qd + 1) * 1024],
                                    em[:, qd, :], ed[:, qd, :], 128, 1024, WQ,
                                )
                                nc.gpsimd.local_scatter(
                                    sg2[:, qd * 1024 : (qd + 1) * 1024],
                                    es[:, qd, :], ed[:, qd, :], 128, 1024, WQ,
                                )

                            # ac = 2qk^T + Mm*(a1+b1) + Sm*(a2+b2)
                            ac = ep.tile([128, N], F32, tag="ac", bufs=1)
                            for n in range(8):
                                s0, s1 = n * 512, (n + 1) * 512
                                aps2 = psb.tile(
                                    [128, 512], F32, tag="apsA", bufs=3
                                )
                                for hg in range(2):
                                    nc.tensor.matmul(
                                        aps2[:],
                                        qt2_t[:, hg, rb * 128 : (rb + 1) * 128],
                                        kt_t[:, hg, s0:s1],
                                        start=(hg == 0),
                                        stop=(hg == 1),
                                    )
                                u = ep.tile([128, 512], BF16, tag="u", bufs=2)
                                w = ep.tile([128, 512], BF16, tag="w", bufs=2)
                                nc.vector.scalar_tensor_tensor(
                                    u[:], b1bc_t[:, s0:s1],
                                    a12_t[:, rb, 0:1], mm[:, s0:s1],
                                    OP.add, OP.mult,
                                )
                                nc.vector.scalar_tensor_tensor(
                                    w[:], b2bc_t[:, s0:s1],
                                    a12_t[:, rb, 1:2], sg2[:, s0:s1],
                                    OP.add, OP.mult,
                                )
                                nc.vector.tensor_tensor(u[:], u[:], w[:], OP.add)
                                nc.vector.tensor_tensor(
                                    ac[:, s0:s1], aps2[:], u[:], OP.add
                                )

                            # exp halves with Z accumulation
                            nc.scalar.activation(
                                expa_t[:, rb, 0 : N // 2],
                                ac[:, 0 : N // 2],
                                AF.Exp,
                                scale=float(SCALE),
                                accum_out=sm_t[:, rb : rb + 1],
                            )
                            nc.scalar.activation(
                                expa_t[:, rb, N // 2 : N],
                                ac[:, N // 2 : N],
                                AF.Exp,
                                scale=float(SCALE),
                                accum_out=sm_t[:, 4 + rb : 5 + rb],
                            )
                            nc.vector.tensor_tensor(
                                sm_t[:, 8 + rb : 9 + rb],
                                sm_t[:, rb : rb + 1],
                                sm_t[:, 4 + rb : 5 + rb],
                                OP.add,
                            )
                            nc.vector.reciprocal(
                                sm_t[:, 8 + rb : 9 + rb], sm_t[:, 8 + rb : 9 + rb]
                            )

                            # u1 = rowsum(P.*Mm) ; u2 = rowsum(P.*Sm) (fused)
                            upr = ep.tile(
                                [128, N], BF16, tag="upr", name="upr", bufs=1
                            )
                            nc.vector.scalar_tensor_tensor(
                                upr[:], expa_t[:, rb, :], 0.0, mm[:],
                                OP.add, OP.mult,
                                accum_out=sm_t[:, 12 + rb : 13 + rb],
                            )
                            nc.vector.scalar_tensor_tensor(
                                upr[:], expa_t[:, rb, :], 0.0, sg2[:],
                                OP.add, OP.mult,
                                accum_out=sm_t[:, 16 + rb : 17 + rb],
                            )

                            # transposes + M matmuls for this row block
                            mps = psb.tile([128, FEAT], F32, tag="mps", bufs=2)
                            for cg in range(8):
                                tp = psb.tile([128, 512], BF16, tag="tp", bufs=3)
                                for j in range(4):
                                    c = 4 * cg + j
                                    nc.tensor.transpose(
                                        tp[:, j * 128 : (j + 1) * 128],
                                        expa_t[:, rb, c * 128 : (c + 1) * 128],
                                        ident_t[:],
                                    )
                                pt = ep.tile([128, 512], BF16, tag="pt", bufs=3)
                                if cg % 2 == 0:
                                    nc.vector.tensor_copy(pt[:], tp[:])
                                else:
                                    nc.scalar.copy(pt[:], tp[:])
                                for j in range(4):
                                    c = 4 * cg + j
                                    nc.tensor.matmul(
                                        mps[:],
                                        pt[:, j * 128 : (j + 1) * 128],
                                        v_t[:, c, :],
                                        start=(c == 0),
                                        stop=(c == 31),
                                    )
                            # (mps + u1*Ev1 + u2*dEv) * rz -> out
                            t1 = ep.tile([128, FEAT], F32, tag="t1", bufs=1)
                            t2 = ep.tile([128, FEAT], F32, tag="t2", bufs=1)
                            mf = ep.tile([128, FEAT], F32, tag="mf", bufs=2)
                            nc.vector.tensor_scalar(
                                t1[:], ev1bc_t[:], sm_t[:, 12 + rb : 13 + rb],
                                None, OP.mult,
                            )
                            nc.vector.tensor_scalar(
                                t2[:], devbc_t[:], sm_t[:, 16 + rb : 17 + rb],
                                None, OP.mult,
                            )
                            nc.vector.tensor_tensor(t1[:], t1[:], t2[:], OP.add)
                            nc.vector.tensor_tensor(t1[:], t1[:], mps[:], OP.add)
                            nc.vector.tensor_scalar(
                                mf[:], t1[:], sm_t[:, 8 + rb : 9 + rb],
                                None, OP.mult,
                            )
                            nc.sync.dma_start(
                                mout[rb * 128 : (rb + 1) * 128, :], mf[:]
                            )

    nc.compile()
    return nc


def _cumcount(keys):
    order = np.argsort(keys, kind="stable")
    ks = keys[order]
    n = len(ks)
    if n == 0:
        return np.zeros(0, np.int64)
    starts = np.r_[0, np.nonzero(ks[1:] != ks[:-1])[0] + 1]
    lens = np.diff(np.r_[starts, n])
    r = np.arange(n) - np.repeat(starts, lens)
    out = np.empty(n, np.int64)
    out[order] = r
    return out


def _prep(inputs):
    x = np.asarray(inputs["x"], np.float32)
    ei = np.asarray(inputs["edge_index"]).astype(np.int64)
    ea = np.asarray(inputs["edge_attr"], np.float32).reshape(-1)
    Wk = np.asarray(inputs["Wk"], np.float32)
    Wq = np.asarray(inputs["Wq"], np.float32)
    Wv = np.asarray(inputs["Wv"], np.float32)
    Ek = np.asarray(inputs["Ek"], np.float32)
    Eq = np.asarray(inputs["Eq"], np.float32)
    Ev = np.asarray(inputs["Ev"], np.float32)
    ib = float(np.asarray(inputs["idx_bias"]).reshape(()))
    im = float(np.asarray(inputs["idx_mult"]).reshape(()))

    assert BF16_NP is not None, "ml_dtypes needed"
    src, dst = ei[0], ei[1]

    # unique cells with multiplicity and sigmoid-sum
    sgm = 1.0 / (1.0 + np.exp(-(ea - ib) * im))
    lin = src * N + dst
    uq, inv, counts = np.unique(lin, return_inverse=True, return_counts=True)
    ssum = np.bincount(inv, weights=sgm, minlength=len(uq)).astype(np.float32)
    u_src = uq // N
    u_dst = uq % N

    c = u_src // RPC
    sl = u_src % RPC
    rb = sl // 128
    p = sl % 128
    qd = u_dst // 1024
    grp = ((c * RB + rb) * 128 + p) * 4 + qd
    rank = _cumcount(grp)
    assert rank.max(initial=0) < WQ, f"WQ overflow: {rank.max()}"

    dste = np.full((NCORES, RB, 4, 128, WQ), -1, np.int16)
    se = np.zeros((NCORES, RB, 4, 128, WQ), BF16_NP)
    me = np.zeros((NCORES, RB, 4, 128, WQ), BF16_NP)
    dste[c, rb, qd, p, rank] = (u_dst - qd * 1024).astype(np.int16)
    se[c, rb, qd, p, rank] = ssum
    me[c, rb, qd, p, rank] = counts

    xT = np.ascontiguousarray(x.T)  # [FEAT, N]
    # full weight matrix [Wk | 2*Wq | Wv] as [4, 128, 1024], then shard cols
    wall = np.concatenate([Wk, 2.0 * Wq, Wv], axis=1).reshape(4, 128, 1024)
    wall16 = wall.astype(BF16_NP)

    shared = {
        "ekbh": np.ascontiguousarray(
            (0.5 * np.stack([Ek[1], Ek[0] - Ek[1]], axis=1)).reshape(2, 128, 2)
        ).astype(BF16_NP),
        "eqb": np.ascontiguousarray(
            np.stack([Eq[1], Eq[0] - Eq[1]], axis=1).reshape(2, 128, 2)
        ).astype(BF16_NP),
        "evr": np.stack([Ev[1], Ev[0] - Ev[1]]).astype(BF16_NP),
        "ident": np.eye(128, dtype=np.float32).astype(BF16_NP),
        "ones1": np.ones((1, 128), BF16_NP),
    }
    in_maps = []
    for cc in range(NCORES):
        m = dict(shared)
        m["xtloc"] = np.ascontiguousarray(
            xT[:, cc * RPC : (cc + 1) * RPC].reshape(4, 128, RPC)
        ).astype(BF16_NP)
        m["wsh"] = np.ascontiguousarray(wall16[:, :, cc * 128 : (cc + 1) * 128])
        m["dste"] = np.ascontiguousarray(dste[cc])
        m["se"] = np.ascontiguousarray(se[cc])
        m["me"] = np.ascontiguousarray(me[cc])
        in_maps.append(m)
    return in_maps


def get_nc(reps=1):
    key = ("v2", reps)
    if key not in _CACHE:
        _CACHE[key] = _build_nc(reps)
    return _CACHE[key]


def kernel(**inputs) -> np.ndarray:
    nc = get_nc()
    in_maps = _prep(inputs)
    res = run_bass_kernel_spmd(nc, in_maps, list(range(NCORES)))
    return np.concatenate(
        [res.results[cc]["mloc"] for cc in range(NCORES)], axis=0
    ).astype(np.float32)
